# revision 12
# baseline (speedup 1.0000x reference)
"""ContMix kernel for TRN2, 8 NeuronCores.

Sharding: (batch b, H-half) -> 8 cores. Each core computes out[b, :, r0:r0+28, :].

Math (per batch b):
  ctx_p = sumpool8x8(ctx)                      [C, 49]   (the /64 is folded into wkt)
  kf    = (Wk/64) @ ctx_p                      [192, 49]
  G     = Wq^T @ kf                            [C, 49]   (so logits = G^T x, q never materialized)
  lg    = G^T @ x                              [49, HW]
  A     = softmax_s(lg)  (no max-sub; |lg| ~ 8)
  dyn   = A @ Wwd^T                            [HW, 25]  (pixel-partition layout, + sum column)
  out[c, n] = sum_j x_patch[c, j, n] * dyn[j, n]
The last step runs on TensorE as banded matmuls: per output-row-pair, a
[128 x 112] banded matrix M (built from dyn via gpsimd local_scatter into
M^T, then PE transpose - no DRAM round trip) contracts against x in
pixel-partition layout (xt, host-pretransposed fp8e3, zero-padded).

v2 vs v1: all DMAs minimized (each costs ~625ns serialized on the HWDGE
device in the cost model); per-pair DRAM round-trip + DMA transposes
replaced by on-chip PE transposes; cx/xn fp16, xt fp8e3, out fp16;
software-pipelined pair loop balancing PE/DVE/Act/Pool; split input/output
DMAs for overlap.
"""

import numpy as np

B, C, H, W = 4, 384, 56, 56
KK, S = 5, 7
NCORES = 8
ROWS = H // 2              # 28 rows per core
NPIX = ROWS * W            # 1568
PADR = ROWS + 4            # 32 padded rows
PADW = 64                  # padded width (>= 56 + 4, and 64-aligned)
WSPACE = PADR * PADW       # 2048 padded pixels
NPAIR = ROWS // 2          # 14 output row-pairs
NCHUNK = PADR // 2         # 16 contraction chunks (2 padded rows = 128 partitions)
D2 = C // 2                # 192
NI = 26                    # scatter idxs (25 taps + 1 sum col, must be even)
MCOLS = 3 * 128            # 384 = contraction space per pair (6 padded rows x 64)
HW = H * W

_cached = {}


def _build_nc():
    import concourse.tile as tile
    from concourse import bacc, mybir, library_config
    from concourse.masks import make_identity

    f32, f16, f8, i16 = (mybir.dt.float32, mybir.dt.float16,
                         mybir.dt.float8e3, mybir.dt.int16)
    nc = bacc.Bacc("TRN2", target_bir_lowering=False, debug=False)

    cx_d = nc.dram_tensor("cx", [128, 3 * HW], f16, kind="ExternalInput")
    xn_d = nc.dram_tensor("xn", [128, 3 * NPIX], f16, kind="ExternalInput")
    xt_d = nc.dram_tensor("xt", [128, NCHUNK * C], f16, kind="ExternalInput")
    wq_d = nc.dram_tensor("wq", [D2, C], f16, kind="ExternalInput")
    wkt_d = nc.dram_tensor("wkt", [128, 3 * D2], f32, kind="ExternalInput")
    wwdt1_d = nc.dram_tensor("wwdt1", [S * S, NI], f32, kind="ExternalInput")
    sidx_d = nc.dram_tensor("sidx", [2 * W, NI], i16, kind="ExternalInput")
    out_d = nc.dram_tensor("out", [128, 3 * NPIX], f16, kind="ExternalOutput")

    # exp/pair chunking: 4 logits chunks aligned to 4-pair groups
    CH = [(0, 448), (448, 448), (896, 448), (1344, 224)]

    with tile.TileContext(nc) as tc:
        with (
            tc.tile_pool(name="big", bufs=1) as big,
            tc.tile_pool(name="wrk", bufs=2) as wrk,
            tc.tile_pool(name="mtp", bufs=2) as mtp,
            tc.tile_pool(name="msb", bufs=2) as msb,
            tc.tile_pool(name="pslg", bufs=2, space="PSUM") as pslg,
            tc.tile_pool(name="psdy", bufs=1, space="PSUM") as psdy,
            tc.tile_pool(name="pstp", bufs=2, space="PSUM") as pstp,
            tc.tile_pool(name="pspo", bufs=3, space="PSUM") as pspo,
        ):
            # gpsimd setup first: off critical path, Pool engine is idle early
            nc.gpsimd.load_library(library_config.local_scatter)
            ident = big.tile([112, 112], f16, tag="ident")
            make_identity(nc, ident[:])

            # ---------------- input DMAs (order = DMA_ENGINES service order) ---
            cx_sb = big.tile([128, 3, HW], f16, tag="cx")
            for u in range(3):
                nc.sync.dma_start(out=cx_sb[:, u, :], in_=cx_d[:, u * HW:(u + 1) * HW])
            wkt_sb = big.tile([128, 3, D2], f32, tag="wkt")
            nc.sync.dma_start(out=wkt_sb[:], in_=wkt_d[:])
            wqA = big.tile([128, C], f16, tag="wqA")
            nc.sync.dma_start(out=wqA[:], in_=wq_d[0:128, :])
            wqB = big.tile([64, C], f16, tag="wqB")
            nc.sync.dma_start(out=wqB[:], in_=wq_d[128:D2, :])
            wwdt1_sb = big.tile([S * S, NI], f32, tag="wwdt1")
            nc.sync.dma_start(out=wwdt1_sb[:], in_=wwdt1_d[:])
            sidx_sb = big.tile([2 * W, NI], i16, tag="sidx")
            nc.sync.dma_start(out=sidx_sb[:], in_=sidx_d[:])
            xn_sb = big.tile([128, 3, NPIX], f16, tag="xn")
            for c0, wdt in CH:
                nc.sync.dma_start(
                    out=xn_sb[:, :, c0:c0 + wdt],
                    in_=xn_d[:].rearrange("p (u n) -> p u n", u=3)[:, :, c0:c0 + wdt])
            xt_sb = big.tile([128, NCHUNK, C], f16, tag="xt")
            nc.sync.dma_start(out=xt_sb[:, 0:8, :], in_=xt_d[:, 0:8 * C])
            nc.sync.dma_start(out=xt_sb[:, 8:16, :], in_=xt_d[:, 8 * C:])

            # ---------------- pooling (sum; /64 folded into wkt) --------------
            ctx_p = big.tile([128, 3, S * S], f32, tag="ctxp")
            kf_ps = [None, None]
            kf_sb = [big.tile([128, S * S], f16, tag="kf0", name="kf0"),
                     big.tile([64, S * S], f16, tag="kf1", name="kf1")]
            for u in range(3):
                p1 = wrk.tile([128, H * S], f16, tag="p1")
                with nc.allow_low_precision(reason="8-elem fp16 partial pool sum"):
                    nc.vector.tensor_reduce(
                        out=p1[:], in_=cx_sb[:, u, :].rearrange("p (h bw j) -> p h bw j", bw=S, j=8),
                        axis=mybir.AxisListType.X, op=mybir.AluOpType.add)
                ap2 = p1[:].rearrange("p (bh i bw) -> p bh i bw", bh=S, i=8)
                ap2 = ap2.rearrange("p bh i bw -> p bh bw i")
                nc.vector.tensor_reduce(
                    out=ctx_p[:, u, :], in_=ap2,
                    axis=mybir.AxisListType.X, op=mybir.AluOpType.add)
                # kf partial for this u chunk (fp32 matmul, tiny): accumulate
                for dc, dw in ((0, 128), (1, 64)):
                    if u == 0:
                        kf_ps[dc] = pslg.tile([dw, S * S], f32, tag="lg", name=f"kf_{dc}")
                    nc.tensor.matmul(kf_ps[dc][:], wkt_sb[:, u, dc * 128:dc * 128 + dw],
                                     ctx_p[:, u, :], start=(u == 0), stop=(u == 2))
            for dc in range(2):
                nc.vector.tensor_copy(kf_sb[dc][:], kf_ps[dc][:])

            # ---------------- G = Wq^T @ kf : [384, 49], fp16 ------------------
            g_sb = big.tile([128, 3, S * S], f16, tag="g")
            for u in range(3):
                g_ps = psdy.tile([128, S * S], f32, tag="dy", name=f"g_{u}")
                nc.tensor.matmul(g_ps[:], wqA[:, u * 128:(u + 1) * 128], kf_sb[0][:],
                                 start=True, stop=False)
                nc.tensor.matmul(g_ps[:], wqB[:, u * 128:(u + 1) * 128], kf_sb[1][:],
                                 start=False, stop=True)
                nc.scalar.copy(g_sb[:, u, :], g_ps[:])

            # ---------------- logits + exp : expa [49, NPIX] fp32 --------------
            expa = big.tile([S * S, NPIX], f32, tag="expa")
            for c0, wdt in CH:
                lg = pslg.tile([S * S, 448], f32, tag="lg")
                for u in range(3):
                    nc.tensor.matmul(lg[:, 0:wdt], g_sb[:, u, :], xn_sb[:, u, c0:c0 + wdt],
                                     start=(u == 0), stop=(u == 2))
                nc.scalar.activation(expa[:, c0:c0 + wdt], lg[:, 0:wdt],
                                     mybir.ActivationFunctionType.Exp)

            # ---------------- software-pipelined pair loop ---------------------
            # iter p: dyn/recip/mul/scatter(p); transposes/mcopy/mms(p-1);
            #         pocopy(p-2); group output DMA
            mt = [None] * NPAIR
            msbt = [None] * NPAIR
            po = [None] * NPAIR
            out_sb = big.tile([128, 3, NPIX], f16, tag="out")
            OUTG = [(4, 0, 560), (9, 560, 560), (13, 1120, 448)]  # (last pair, off, wdt)
            for p in range(NPAIR + 2):
                if p < NPAIR:
                    dyn_ps = psdy.tile([2 * W, NI], f32, tag="dy")
                    nc.tensor.matmul(dyn_ps[:], expa[:, p * 112:(p + 1) * 112],
                                     wwdt1_sb[:], start=True, stop=True)
                    rec = wrk.tile([2 * W, 1], f32, tag="rec")
                    nc.vector.reciprocal(rec[:], dyn_ps[:, 25:26])
                    d16 = wrk.tile([2 * W, NI], f16, tag="d16")
                    nc.scalar.activation(d16[:], dyn_ps[:],
                                         mybir.ActivationFunctionType.Copy, scale=rec[:])
                    mt[p] = mtp.tile([2 * W, MCOLS], f16, tag="mt", name=f"mt_{p}")
                    nc.gpsimd.local_scatter(mt[p][:], d16[:], sidx_sb[:],
                                            channels=2 * W, num_elems=MCOLS, num_idxs=NI)
                if 1 <= p <= NPAIR:
                    q = p - 1
                    tp = pstp.tile([128, 3, 112], f16, tag="tp")
                    for t3 in range(3):
                        nc.tensor.transpose(tp[:, t3, :], mt[q][:, t3 * 128:(t3 + 1) * 128],
                                            ident[:])
                    mt[q] = None
                    msbt[q] = msb.tile([128, 3, 112], f16, tag="m", name=f"m_{q}")
                    nc.vector.tensor_copy(msbt[q][:], tp[:])
                    po[q] = pspo.tile([128, 3, 112], f32, tag="po", name=f"po_{q}")
                    for cc in range(3):
                        for trel in range(3):
                            nc.tensor.matmul(po[q][:, cc, :],
                                             xt_sb[:, q + trel, cc * 128:(cc + 1) * 128],
                                             msbt[q][:, trel, :],
                                             start=(trel == 0), stop=(trel == 2))
                if 2 <= p:
                    q2 = p - 2
                    dst = out_sb[:, :, q2 * 112:(q2 + 1) * 112]
                    if q2 % 2 == 0:
                        nc.vector.tensor_copy(dst, po[q2][:])
                    else:
                        nc.scalar.copy(dst, po[q2][:])
                    po[q2] = None
                    msbt[q2] = None
                    for gl, goff, gwdt in OUTG:
                        if q2 == gl:
                            nc.sync.dma_start(
                                out=out_d[:].rearrange("p (u n) -> p u n", u=3)[:, :, goff:goff + gwdt],
                                in_=out_sb[:, :, goff:goff + gwdt])
    nc.finalize()
    return nc


def _static_inputs():
    # scatter index table: pixel n = hl*56 + w ; tap j = 5*di + dj
    sidx = np.full((2 * W, NI), -1, np.int16)
    for hl in range(2):
        for w in range(W):
            for di in range(KK):
                for dj in range(KK):
                    sidx[hl * W + w, 5 * di + dj] = (hl + di) * PADW + w + dj
    return sidx


def _prep(x, ctx, Wq, Wk, Wwd):
    import ml_dtypes
    f8 = ml_dtypes.float8_e3m4
    sidx = _static_inputs()
    wkt = (Wk.T / 64.0).astype(np.float32).reshape(3, 128, D2).transpose(1, 0, 2)
    wkt = np.ascontiguousarray(wkt.reshape(128, 3 * D2))
    wwdt1 = np.concatenate([Wwd.T, np.ones((S * S, 1), np.float32)], axis=1).astype(np.float32)
    wq = np.ascontiguousarray(Wq).astype(np.float16)
    in_maps = []
    for core in range(NCORES):
        b, half = core // 2, core % 2
        r0 = half * ROWS
        xn = x[b].reshape(3, 128, H, W)[:, :, r0:r0 + ROWS, :]
        xn = np.ascontiguousarray(xn.transpose(1, 0, 2, 3).reshape(128, 3 * NPIX)).astype(np.float16)
        xp = np.zeros((PADR, PADW, C), np.float32)
        lo, hi = max(0, r0 - 2), min(H, r0 + ROWS + 2)
        xp[lo - (r0 - 2):hi - (r0 - 2), 2:2 + W, :] = np.transpose(x[b, :, lo:hi, :], (1, 2, 0))
        xt = xp.reshape(NCHUNK, 128, C).transpose(1, 0, 2).reshape(128, NCHUNK * C)
        xt = np.ascontiguousarray(xt).astype(np.float16)
        cx = np.ascontiguousarray(ctx[b].reshape(3, 128, HW).transpose(1, 0, 2)
                                  .reshape(128, 3 * HW)).astype(np.float16)
        in_maps.append(dict(xn=xn, xt=xt, cx=cx, wq=wq, wkt=wkt, wwdt1=wwdt1, sidx=sidx))
    return in_maps


def kernel(x, ctx, Wq, Wk, Wwd, _trace=False):
    from concourse.bass_utils import run_bass_kernel_spmd

    x, ctx = np.asarray(x), np.asarray(ctx)
    Wq, Wk, Wwd = np.asarray(Wq), np.asarray(Wk), np.asarray(Wwd)
    if "nc" not in _cached:
        _cached["nc"] = _build_nc()
    in_maps = _prep(x, ctx, Wq, Wk, Wwd)
    res = run_bass_kernel_spmd(_cached["nc"], in_maps, list(range(NCORES)), trace=_trace)
    _cached["last_result"] = res
    out = np.empty((B, C, H, W), np.float32)
    for core in range(NCORES):
        b, half = core // 2, core % 2
        r0 = half * ROWS
        o = res.results[core]["out"].astype(np.float32).reshape(128, 3, ROWS, W)
        out[b, :, r0:r0 + ROWS, :] = o.transpose(1, 0, 2, 3).reshape(C, ROWS, W)
    return out


# revision 19
# speedup vs baseline: 1.4088x; 1.4088x over previous
"""ContMix kernel for TRN2, 8 NeuronCores.

Sharding: (batch b, H-half) -> 8 cores. Each core computes out[b, :, r0:r0+28, :].

Math (per batch b):
  ctx_p = sumpool8x8(ctx)                      [C, 49]   (the /64 is folded into wkt)
  kf    = (Wk/64) @ ctx_p                      [192, 49]
  G     = Wq^T @ kf                            [C, 49]   (so logits = G^T x, q never materialized)
  lg    = G^T @ x                              [49, HW]
  A     = softmax_s(lg)  (no max-sub; |lg| ~ 8)
  dyn   = A @ Wwd^T                            [HW, 25]  (pixel-partition layout, + sum column)
  out[c, n] = sum_j x_patch[c, j, n] * dyn[j, n]
The last step runs on TensorE as banded matmuls: per output-row-pair, a
[128 x 112] banded matrix M (built from dyn via gpsimd local_scatter into
M^T, then PE transpose - no DRAM round trip) contracts against x in
pixel-partition layout (xt, host-pretransposed fp8e3, zero-padded).

v2 vs v1: all DMAs minimized (each costs ~625ns serialized on the HWDGE
device in the cost model); per-pair DRAM round-trip + DMA transposes
replaced by on-chip PE transposes; cx/xn fp16, xt fp8e3, out fp16;
software-pipelined pair loop balancing PE/DVE/Act/Pool; split input/output
DMAs for overlap.
"""

import numpy as np

B, C, H, W = 4, 384, 56, 56
KK, S = 5, 7
NCORES = 8
ROWS = H // 2              # 28 rows per core
NPIX = ROWS * W            # 1568
PADR = ROWS + 4            # 32 padded rows
PADW = 64                  # padded width (>= 56 + 4, and 64-aligned)
WSPACE = PADR * PADW       # 2048 padded pixels
NPAIR = ROWS // 2          # 14 output row-pairs
NCHUNK = PADR // 2         # 16 contraction chunks (2 padded rows = 128 partitions)
D2 = C // 2                # 192
NI = 26                    # scatter idxs (25 taps + 1 sum col, must be even)
MCOLS = 3 * 128            # 384 = contraction space per pair (6 padded rows x 64)
HW = H * W

_cached = {}


def _build_nc():
    import concourse.tile as tile
    from concourse import bacc, mybir, library_config
    from concourse.masks import make_identity

    f32, f16, f8, i16 = (mybir.dt.float32, mybir.dt.float16,
                         mybir.dt.float8e3, mybir.dt.int16)
    nc = bacc.Bacc("TRN2", target_bir_lowering=False, debug=False)

    cx_d = nc.dram_tensor("cx", [128, 3 * HW], f16, kind="ExternalInput")
    xn_d = nc.dram_tensor("xn", [128, 3 * NPIX], f16, kind="ExternalInput")
    xt_d = nc.dram_tensor("xt", [128, NCHUNK * C], f16, kind="ExternalInput")
    wq_d = nc.dram_tensor("wq", [D2, C], f16, kind="ExternalInput")
    wkt_d = nc.dram_tensor("wkt", [128, 3 * D2], f16, kind="ExternalInput")
    wwdt1_d = nc.dram_tensor("wwdt1", [S * S, NI], f32, kind="ExternalInput")
    sidx_d = nc.dram_tensor("sidx", [2 * W, NI], i16, kind="ExternalInput")
    out_d = nc.dram_tensor("out", [128, 3 * NPIX], f16, kind="ExternalOutput")

    # logits/exp chunks aligned to pair boundaries; small first chunk to
    # prime the pair pipeline early
    CH = [(0, 112), (112, 336), (448, 448), (896, 448), (1344, 224)]
    XNCH = [(0, 448), (448, 448), (896, 448), (1344, 224)]

    with tile.TileContext(nc) as tc:
        with (
            tc.tile_pool(name="big", bufs=1) as big,
            tc.tile_pool(name="wrk", bufs=2) as wrk,
            tc.tile_pool(name="mtp", bufs=3) as mtp,
            tc.tile_pool(name="msb", bufs=2) as msb,
            tc.tile_pool(name="psA", bufs=2, space="PSUM") as psA,
            tc.tile_pool(name="psdy", bufs=2, space="PSUM") as psdy,
            tc.tile_pool(name="psB", bufs=3, space="PSUM") as psB,
        ):
            # gpsimd setup first: off critical path, Pool engine is idle early
            nc.gpsimd.load_library(library_config.local_scatter)
            ident = big.tile([112, 112], f16, tag="ident")
            make_identity(nc, ident[:])

            # ---------------- input DMAs (order = DMA_ENGINES service order) ---
            cx_sb = big.tile([128, 3, HW], f16, tag="cx")
            for u in range(3):
                nc.sync.dma_start(out=cx_sb[:, u, :], in_=cx_d[:, u * HW:(u + 1) * HW])
            wkt_sb = big.tile([128, 3, D2], f16, tag="wkt")
            nc.sync.dma_start(out=wkt_sb[:], in_=wkt_d[:])
            wqA = big.tile([128, C], f16, tag="wqA")
            nc.sync.dma_start(out=wqA[:], in_=wq_d[0:128, :])
            wqB = big.tile([64, C], f16, tag="wqB")
            nc.sync.dma_start(out=wqB[:], in_=wq_d[128:D2, :])
            wwdt1_sb = big.tile([S * S, NI], f32, tag="wwdt1")
            nc.sync.dma_start(out=wwdt1_sb[:], in_=wwdt1_d[:])
            sidx_sb = big.tile([2 * W, NI], i16, tag="sidx")
            nc.sync.dma_start(out=sidx_sb[:], in_=sidx_d[:])
            xn_sb = big.tile([128, 3, NPIX], f16, tag="xn")
            for c0, wdt in XNCH:
                nc.sync.dma_start(
                    out=xn_sb[:, :, c0:c0 + wdt],
                    in_=xn_d[:].rearrange("p (u n) -> p u n", u=3)[:, :, c0:c0 + wdt])
            xt_sb = big.tile([128, NCHUNK, C], f16, tag="xt")
            nc.sync.dma_start(out=xt_sb[:, 0:8, :], in_=xt_d[:, 0:8 * C])
            nc.sync.dma_start(out=xt_sb[:, 8:16, :], in_=xt_d[:, 8 * C:])

            # ---------------- pooling (sum; /64 folded into wkt) --------------
            # pairwise add trees on DVE (tensor_tensor hits the 2x/4x packed
            # fp16 modes; tensor_reduce never does)
            ctx_p = big.tile([128, 3, S * S], f16, tag="ctxp")
            kf_ps = [None, None]
            kf_sb = [big.tile([128, S * S], f16, tag="kf0", name="kf0"),
                     big.tile([64, S * S], f16, tag="kf1", name="kf1")]
            add = mybir.AluOpType.add
            for u in range(3):
                t1 = wrk.tile([128, H * S, 4], f16, tag="t1")
                t2 = wrk.tile([128, H * S, 2], f16, tag="t2")
                p1 = wrk.tile([128, H * S], f16, tag="p1")
                with nc.allow_low_precision(reason="fp16 pool partial sums"):
                    v = cx_sb[:, u, :].rearrange("p (hb j) -> p hb j", j=8)
                    nc.vector.tensor_tensor(t1[:], v[:, :, 0:4], v[:, :, 4:8], op=add)
                    nc.vector.tensor_tensor(t2[:], t1[:, :, 0:2], t1[:, :, 2:4], op=add)
                    nc.vector.tensor_tensor(p1[:], t2[:, :, 0], t2[:, :, 1], op=add)
                    # stage 2: sum 8 rows within each bin; layout (bh i bw)
                    s1 = p1[:].rearrange("p (bh i bw) -> p bh i bw", bh=S, i=8)
                    u1 = wrk.tile([128, S, 4, S], f16, tag="u1")
                    u2 = wrk.tile([128, S, 2, S], f16, tag="u2")
                    nc.vector.tensor_tensor(u1[:], s1[:, :, 0:4, :], s1[:, :, 4:8, :], op=add)
                    nc.vector.tensor_tensor(u2[:], u1[:, :, 0:2, :], u1[:, :, 2:4, :], op=add)
                    nc.vector.tensor_tensor(ctx_p[:, u, :].rearrange("p (bh bw) -> p bh bw", bh=S),
                                            u2[:, :, 0, :], u2[:, :, 1, :], op=add)
                # kf partial for this u chunk (fp16 matmul, tiny): accumulate
                for dc, dw in ((0, 128), (1, 64)):
                    if u == 0:
                        kf_ps[dc] = psB.tile([dw, S * S], f32, tag="po", name=f"kf_{dc}")
                    nc.tensor.matmul(kf_ps[dc][:], wkt_sb[:, u, dc * 128:dc * 128 + dw],
                                     ctx_p[:, u, :], start=(u == 0), stop=(u == 2))
            for dc in range(2):
                nc.vector.tensor_copy(kf_sb[dc][:], kf_ps[dc][:])

            # ---------------- G = Wq^T @ kf : [384, 49], fp16 ------------------
            g_sb = big.tile([128, 3, S * S], f16, tag="g")
            for u in range(3):
                g_ps = psdy.tile([128, S * S], f32, tag="dy", name=f"g_{u}")
                nc.tensor.matmul(g_ps[:], wqA[:, u * 128:(u + 1) * 128], kf_sb[0][:],
                                 start=True, stop=False)
                nc.tensor.matmul(g_ps[:], wqB[:, u * 128:(u + 1) * 128], kf_sb[1][:],
                                 start=False, stop=True)
                nc.scalar.copy(g_sb[:, u, :], g_ps[:])

            # ---------------- logits + exp : expa [49, NPIX] fp32 --------------
            expa = big.tile([S * S, NPIX], f32, tag="expa")
            for c0, wdt in CH:
                lg = psA.tile([S * S, 448], f32, tag="lg")
                for u in range(3):
                    nc.tensor.matmul(lg[:, 0:wdt], g_sb[:, u, :], xn_sb[:, u, c0:c0 + wdt],
                                     start=(u == 0), stop=(u == 2))
                nc.scalar.activation(expa[:, c0:c0 + wdt], lg[:, 0:wdt],
                                     mybir.ActivationFunctionType.Exp)

            # ---------------- software-pipelined pair loop (stagger 2/3) -------
            # iter p: dyn/div/scatter(p); transposes/mcopy/mms(p-2);
            #         pocopy(p-3); group output DMA
            mt = [None] * NPAIR
            msbt = [None] * NPAIR
            po = [None] * NPAIR
            out_sb = big.tile([128, 3, NPIX], f16, tag="out")
            OUTG = [(4, 0, 560), (8, 560, 448), (11, 1008, 336), (13, 1344, 224)]
            for p in range(NPAIR + 3):
                if p < NPAIR:
                    dyn_ps = psdy.tile([2 * W, NI], f32, tag="dy")
                    nc.tensor.matmul(dyn_ps[:], expa[:, p * 112:(p + 1) * 112],
                                     wwdt1_sb[:], start=True, stop=True)
                    rec = wrk.tile([2 * W, 1], f32, tag="rec")
                    nc.vector.reciprocal(rec[:], dyn_ps[:, 25:26])
                    d16 = wrk.tile([2 * W, NI], f16, tag="d16")
                    nc.scalar.activation(d16[:], dyn_ps[:],
                                         mybir.ActivationFunctionType.Copy, scale=rec[:])
                    mt[p] = mtp.tile([2 * W, MCOLS], f16, tag="mt", name=f"mt_{p}")
                    nc.gpsimd.local_scatter(mt[p][:], d16[:], sidx_sb[:],
                                            channels=2 * W, num_elems=MCOLS, num_idxs=NI)
                if 2 <= p < NPAIR + 2:
                    q = p - 2
                    tp = psA.tile([128, 3, 112], f16, tag="lg", name=f"tp_{q}")
                    for t3 in range(3):
                        nc.tensor.transpose(tp[:, t3, :], mt[q][:, t3 * 128:(t3 + 1) * 128],
                                            ident[:])
                    mt[q] = None
                    msbt[q] = msb.tile([128, 3, 112], f16, tag="m", name=f"m_{q}")
                    nc.vector.tensor_copy(msbt[q][:], tp[:])
                    po[q] = psB.tile([128, 3, 112], f32, tag="po", name=f"po_{q}")
                    for cc in range(3):
                        for trel in range(3):
                            nc.tensor.matmul(po[q][:, cc, :],
                                             xt_sb[:, q + trel, cc * 128:(cc + 1) * 128],
                                             msbt[q][:, trel, :],
                                             start=(trel == 0), stop=(trel == 2))
                if 3 <= p:
                    q2 = p - 3
                    dst = out_sb[:, :, q2 * 112:(q2 + 1) * 112]
                    if q2 % 2 == 0:
                        nc.vector.tensor_copy(dst, po[q2][:])
                    else:
                        nc.scalar.copy(dst, po[q2][:])
                    po[q2] = None
                    msbt[q2] = None
                    for gl, goff, gwdt in OUTG:
                        if q2 == gl:
                            nc.sync.dma_start(
                                out=out_d[:].rearrange("p (u n) -> p u n", u=3)[:, :, goff:goff + gwdt],
                                in_=out_sb[:, :, goff:goff + gwdt])
    nc.finalize()
    return nc


def _static_inputs():
    # scatter index table: pixel n = hl*56 + w ; tap j = 5*di + dj
    sidx = np.full((2 * W, NI), -1, np.int16)
    for hl in range(2):
        for w in range(W):
            for di in range(KK):
                for dj in range(KK):
                    sidx[hl * W + w, 5 * di + dj] = (hl + di) * PADW + w + dj
    return sidx


def _prep(x, ctx, Wq, Wk, Wwd):
    import ml_dtypes
    f8 = ml_dtypes.float8_e3m4
    sidx = _static_inputs()
    wkt = (Wk.T / 64.0).reshape(3, 128, D2).transpose(1, 0, 2)
    wkt = np.ascontiguousarray(wkt.reshape(128, 3 * D2)).astype(np.float16)
    wwdt1 = np.concatenate([Wwd.T, np.ones((S * S, 1), np.float32)], axis=1).astype(np.float32)
    wq = np.ascontiguousarray(Wq).astype(np.float16)
    in_maps = []
    for core in range(NCORES):
        b, half = core // 2, core % 2
        r0 = half * ROWS
        xn = x[b].reshape(3, 128, H, W)[:, :, r0:r0 + ROWS, :]
        xn = np.ascontiguousarray(xn.transpose(1, 0, 2, 3).reshape(128, 3 * NPIX)).astype(np.float16)
        xp = np.zeros((PADR, PADW, C), np.float32)
        lo, hi = max(0, r0 - 2), min(H, r0 + ROWS + 2)
        xp[lo - (r0 - 2):hi - (r0 - 2), 2:2 + W, :] = np.transpose(x[b, :, lo:hi, :], (1, 2, 0))
        xt = xp.reshape(NCHUNK, 128, C).transpose(1, 0, 2).reshape(128, NCHUNK * C)
        xt = np.ascontiguousarray(xt).astype(np.float16)
        cx = np.ascontiguousarray(ctx[b].reshape(3, 128, HW).transpose(1, 0, 2)
                                  .reshape(128, 3 * HW)).astype(np.float16)
        in_maps.append(dict(xn=xn, xt=xt, cx=cx, wq=wq, wkt=wkt, wwdt1=wwdt1, sidx=sidx))
    return in_maps


def kernel(x, ctx, Wq, Wk, Wwd, _trace=False):
    from concourse.bass_utils import run_bass_kernel_spmd

    x, ctx = np.asarray(x), np.asarray(ctx)
    Wq, Wk, Wwd = np.asarray(Wq), np.asarray(Wk), np.asarray(Wwd)
    if "nc" not in _cached:
        _cached["nc"] = _build_nc()
    in_maps = _prep(x, ctx, Wq, Wk, Wwd)
    res = run_bass_kernel_spmd(_cached["nc"], in_maps, list(range(NCORES)), trace=_trace)
    _cached["last_result"] = res
    out = np.empty((B, C, H, W), np.float32)
    for core in range(NCORES):
        b, half = core // 2, core % 2
        r0 = half * ROWS
        o = res.results[core]["out"].astype(np.float32).reshape(128, 3, ROWS, W)
        out[b, :, r0:r0 + ROWS, :] = o.transpose(1, 0, 2, 3).reshape(C, ROWS, W)
    return out
